# revision 10
# baseline (speedup 1.0000x reference)
"""Trainium2 Bass kernel for nn_CompositionBlock (gnn_message_passing).

Reference semantics (per batch b, S=2048 tokens, T=128 dims):
    h        = tanh(token)                               # [S, T]
    val[s,t] = sum_pq token[s,p] W[t,p,q] h[s,q] + b_comp[t]
    act      = tanh(val)
    delta    = w_red[s] * (act[s,t] - tanh(b_comp)[t])
    out[i,t] = sum_s w_red[s]*tanh(b_comp)[t] + b_red
               + sum_{s: heads[s]==i} delta[s,t]

Sharding: data-parallel over batch B=8 -> one batch per NeuronCore; W and
the small vectors replicated. No collectives.

HW calibration (measured): PE matmul ~0.42ns per rhs column (fp16),
independent of out partitions; DVE ~0.55ns/col regardless of dtype (fp16
packing and fp8 DoubleRow give no real wins; fp8 DVE output HALVES DVE
rate). So: everything fp16, PE is the wall at ~235us, DVE ~176us under it,
Scalar nearly idle and takes all per-partition-scalar work.

Device algorithm per core (all matmuls fp16 in / f32 psum accum):
  MM1 (PE):  A[q, (i,s)] = W_ti[p,q].T @ tokT[p, s]  (t-pair i=0,1)
  TT  (DVE): Z = A * hT   (f32 PSUM in, fp16 out; the only DVE pass)
  MM2 (PE):  valT[t, s] += E_t.T @ Z_i  (staircase ones-column selector)
  ACT (Scalar): actT = tanh(valT + b_comp[t]); dT = actT - tanh(b_comp)
  dj = DMA-xbar transpose of dT (sync/gpsimd queues); dw = w_red[j] * dj
  (Scalar per-partition scale).
  MM3 (PE):  outT[t,i] += dw_j.T @ MT_j with host-baked one-hot MT.
  Chunks 0-2 of outT accumulate INLINE during the main loop (3 spare PSUM
  banks); chunk 3 runs in a short tail. outT += base (Scalar); DMA out.
Host transposes outT -> out per batch at gather time.
"""

import os
from contextlib import ExitStack

import numpy as np

import concourse.bass as bass
import concourse.tile as tile
from concourse import bacc, mybir
from concourse.bass_utils import run_bass_kernel_spmd

B, S, T = 8, 2048, 128
P = 128
N_CORES = 8
NST = S // P      # 16 s-tiles of 128
NSG = S // 512    # 4 s-groups of 512
NPAIR = T // 2    # 64 t-pairs
F32 = mybir.dt.float32
F16 = mybir.dt.float16
AF = mybir.ActivationFunctionType
ALU = mybir.AluOpType

_NC_CACHE = {}


def build_nc():
    nc = bacc.Bacc("TRN2", target_bir_lowering=False, debug=False,
                   num_devices=N_CORES)

    tokT_d = nc.dram_tensor("tokT", [P, S], F16, kind="ExternalInput").ap()
    w16_d = nc.dram_tensor("w16", [P, T * T], F16, kind="ExternalInput").ap()
    mts_d = nc.dram_tensor("mts", [S, S], F16, kind="ExternalInput").ap()
    bcompT_d = nc.dram_tensor("bcompT", [T, 1], F32, kind="ExternalInput").ap()
    nbasev_d = nc.dram_tensor("nbasev", [T, 1], F32, kind="ExternalInput").ap()
    baseT_d = nc.dram_tensor("baseT", [T, 1], F32, kind="ExternalInput").ap()
    wred_d = nc.dram_tensor("wred", [P, NST], F32, kind="ExternalInput").ap()
    outT_d = nc.dram_tensor("outT", [T, S], F32, kind="ExternalOutput").ap()

    with tile.TileContext(nc) as tc:
        _body(tc, tokT_d, w16_d, mts_d, bcompT_d, nbasev_d, baseT_d,
              wred_d, outT_d)
    nc.compile()
    return nc


def _body(tc, tokT_d, w16_d, mts_d, bcompT_d, nbasev_d, baseT_d,
          wred_d, outT_d):
    nc = tc.nc
    with ExitStack() as ctx:
        const = ctx.enter_context(tc.tile_pool(name="const", bufs=1))
        zpool = ctx.enter_context(tc.tile_pool(name="zpool", bufs=6))
        a16p = ctx.enter_context(tc.tile_pool(name="a16p", bufs=6))
        spool = ctx.enter_context(tc.tile_pool(name="spool", bufs=2))
        djp = ctx.enter_context(tc.tile_pool(name="djp", bufs=3))
        dwp = ctx.enter_context(tc.tile_pool(name="dwp", bufs=1))

        # staircase first: Q must be ready before the first MM2, and gpsimd
        # engine instructions queue behind its DMA drains otherwise.
        Q = const.tile([P, 2 * P - 1], F16)
        nc.gpsimd.memset(Q[:], 0.0)
        nc.gpsimd.memset(Q[:, P - 1: P], 1.0)

        # tokT on the (empty) sync queue so hT2+MM1 start immediately;
        # the 4MB of W alone on gpsimd, chunk 0 first.
        tokTs = []
        for g in range(NSG):
            tokT_g = const.tile([P, 512], F16, tag=f"tokT{g}", name=f"tokT{g}")
            nc.sync.dma_start(out=tokT_g[:],
                              in_=tokT_d[:, 512 * g: 512 * (g + 1)])
            tokTs.append(tokT_g)
        w_tiles = []
        for wc in range(8):
            wt = const.tile([P, 2048], F16, tag=f"w{wc}", name=f"w{wc}")
            nc.gpsimd.dma_start(out=wt[:],
                                in_=w16_d[:, 2048 * wc: 2048 * (wc + 1)])
            w_tiles.append(wt)

        # small consts on sync queue (tiny, land immediately)
        bcompT_sb = const.tile([T, 1], F32)
        nc.sync.dma_start(out=bcompT_sb[:], in_=bcompT_d[:])
        nbasev_sb = const.tile([T, 1], F32)
        nc.sync.dma_start(out=nbasev_sb[:], in_=nbasev_d[:])
        baseT_sb = const.tile([T, 1], F32)
        nc.sync.dma_start(out=baseT_sb[:], in_=baseT_d[:])
        wred_sb = const.tile([P, NST], F32)
        nc.sync.dma_start(out=wred_sb[:], in_=wred_d[:])

        # one-hot scatter matrices (host-baked): MT[j, i] = (heads[j] == i).
        # First 4 loaded up front (needed by inline MM3 after group 0);
        # the rest issue inside the loop to spread HBM traffic.
        mts = []
        for j in range(NST):
            mt_j = const.tile([P, S], F16, tag=f"mt{j}", name=f"mt{j}")
            mts.append(mt_j)
        for j in range(4):
            nc.sync.dma_start(out=mts[j][:], in_=mts_d[P * j: P * (j + 1), :])

        hT2s = []
        for g in range(NSG):
            hT2 = const.tile([P, 1024], F16, tag=f"hT2_{g}", name=f"hT2_{g}")
            nc.scalar.activation(hT2[:, 0:512], tokTs[g][:], AF.Tanh)
            nc.scalar.activation(hT2[:, 512:1024], tokTs[g][:], AF.Tanh)
            hT2s.append(hT2)

        # ---- main loop; OT chunks 0-2 accumulate inline ----
        outT_sb = const.tile([P, S], F32)
        dws = []
        with tc.tile_pool(name="psumA", bufs=2, space="PSUM") as psumA, \
             tc.tile_pool(name="psumV", bufs=1, space="PSUM") as psumV, \
             tc.tile_pool(name="psumO", bufs=1, space="PSUM") as psumO:
            OT012 = psumO.tile([P, 3, 512], F32, space="PSUM", tag="OT012",
                               name="OT012")
            OTs = [OT012[:, c, :] for c in range(3)]
            for g in range(NSG):
                hT2 = hT2s[g]
                V = psumV.tile([P, 512], F32, space="PSUM", tag="V", name="V")
                for tp in range(NPAIR):
                    t0, t1 = 2 * tp, 2 * tp + 1
                    A = psumA.tile([P, 1024], F32, space="PSUM", tag="A",
                                   name="A")
                    w_t0 = w_tiles[t0 // 16][:, T * (t0 % 16): T * (t0 % 16 + 1)]
                    w_t1 = w_tiles[t1 // 16][:, T * (t1 % 16): T * (t1 % 16 + 1)]
                    nc.tensor.matmul(A[:, 0:512], lhsT=w_t0,
                                     rhs=tokTs[g][:], start=True, stop=True)
                    nc.tensor.matmul(A[:, 512:1024], lhsT=w_t1,
                                     rhs=tokTs[g][:], start=True, stop=True)
                    Z = zpool.tile([P, 1024], F16, tag="Z", name="Z")
                    if tp % 4 != 3:
                        # Scalar downcast to fp16 SBUF: DVE reads PSUM f32 at
                        # half rate (~1135ns vs ~650ns), so 3/4 of pairs go
                        # through the idle Scalar engine.
                        A16 = a16p.tile([P, 1024], F16, tag="A16", name="A16")
                        nc.scalar.activation(A16[:], A[:], AF.Copy)
                        nc.vector.tensor_tensor(out=Z[:], in0=A16[:],
                                                in1=hT2[:], op=ALU.mult)
                    else:
                        nc.vector.tensor_tensor(out=Z[:], in0=A[:],
                                                in1=hT2[:], op=ALU.mult)
                    nc.tensor.matmul(V[:],
                                     lhsT=Q[:, P - 1 - t0: 2 * P - 1 - t0],
                                     rhs=Z[:, 0:512], start=(tp == 0),
                                     stop=False)
                    nc.tensor.matmul(V[:],
                                     lhsT=Q[:, P - 1 - t1: 2 * P - 1 - t1],
                                     rhs=Z[:, 512:1024], start=False,
                                     stop=(tp == NPAIR - 1))
                actT = spool.tile([P, 512], F16, tag="actT", name="actT")
                nc.scalar.activation(actT[:], V[:], AF.Tanh, bias=bcompT_sb[:])
                dT = spool.tile([P, 512], F16, tag="dT", name="dT")
                nc.vector.tensor_scalar_add(dT[:], actT[:], nbasev_sb[:])
                for k in range(4):
                    j = 4 * g + k
                    dj = djp.tile([P, P], F16, tag="dj", name="dj")
                    nc.sync.dma_start_transpose(out=dj[:],
                                                in_=dT[:, P * k: P * (k + 1)])
                    dw_j = dwp.tile([P, P], F16, tag=f"dw{j}", name=f"dw{j}")
                    nc.vector.tensor_scalar_mul(dw_j[:], dj[:],
                                                wred_sb[:, j: j + 1])
                    dws.append(dw_j)
                    # inline MM3 for chunks 0-2
                    for c in range(3):
                        nc.tensor.matmul(OTs[c], lhsT=dw_j[:],
                                         rhs=mts[j][:, 512 * c: 512 * (c + 1)],
                                         start=(j == 0), stop=(j == NST - 1))
                    # spread the remaining one-hot loads through the loop
                    if g < 3:
                        jn = 4 * (g + 1) + k
                        nc.sync.dma_start(out=mts[jn][:],
                                          in_=mts_d[P * jn: P * (jn + 1), :])
            for c in range(3):
                cs = slice(512 * c, 512 * (c + 1))
                nc.vector.tensor_scalar_add(outT_sb[:, cs], OTs[c], baseT_sb[:])
                nc.sync.dma_start(out=outT_d[:, cs], in_=outT_sb[:, cs])

        # ---- tail: chunk 3 of the scatter ----
        with tc.tile_pool(name="psumO2", bufs=1, space="PSUM") as psumO2:
            OT3 = psumO2.tile([P, 512], F32, space="PSUM", tag="OT3",
                              name="OT3")
            for j in range(NST):
                nc.tensor.matmul(OT3[:], lhsT=dws[j][:],
                                 rhs=mts[j][:, 1536:2048],
                                 start=(j == 0), stop=(j == NST - 1))
            nc.vector.tensor_scalar_add(outT_sb[:, 1536:2048], OT3[:], baseT_sb[:])
            nc.sync.dma_start(out=outT_d[:, 1536:2048],
                              in_=outT_sb[:, 1536:2048])


def _prep_inputs(token_embeddings, dep_heads, W_comp, b_comp, w_red, b_red):
    """Host-side sharding + layout prep. One in_map per core (= per batch)."""
    token = np.asarray(token_embeddings, np.float32)
    heads = np.asarray(dep_heads, np.int32)
    W = np.asarray(W_comp, np.float32)
    w16 = np.ascontiguousarray(
        W.transpose(1, 0, 2).reshape(P, T * T).astype(np.float16))
    bc = np.asarray(b_comp, np.float32)
    bcompT = np.ascontiguousarray(bc.reshape(T, 1))
    basev = np.tanh(bc)
    nbasev = np.ascontiguousarray(-basev.reshape(T, 1)).astype(np.float32)
    w = np.asarray(w_red, np.float32)[0]
    baseT = (w.sum() * basev + np.asarray(b_red, np.float32)[0])
    baseT = np.ascontiguousarray(baseT.reshape(T, 1)).astype(np.float32)
    wred = np.ascontiguousarray(w.reshape(NST, P).T)

    in_maps = []
    for b in range(B):
        mts = np.zeros((S, S), np.float16)
        mts[np.arange(S), heads[b]] = 1.0
        in_maps.append({
            "tokT": np.ascontiguousarray(token[b].T.astype(np.float16)),
            "w16": w16,
            "mts": mts,
            "bcompT": bcompT,
            "nbasev": nbasev,
            "baseT": baseT,
            "wred": wred,
        })
    return in_maps


def kernel(**inputs):
    if "nc" not in _NC_CACHE:
        _NC_CACHE["nc"] = build_nc()
    nc = _NC_CACHE["nc"]
    in_maps = _prep_inputs(
        inputs["token_embeddings"], inputs["dep_heads"], inputs["W_comp"],
        inputs["b_comp"], inputs["w_red"], inputs["b_red"])
    res = run_bass_kernel_spmd(nc, in_maps, core_ids=list(range(N_CORES)))
    out = np.empty((B, S, T), np.float32)
    for b in range(B):
        out[b] = res.results[b]["outT"].T
    return out


# revision 11
# speedup vs baseline: 1.3768x; 1.3768x over previous
"""Trainium2 Bass kernel for nn_CompositionBlock (gnn_message_passing).

Reference semantics (per batch b, S=2048 tokens, T=128 dims):
    h        = tanh(token)                               # [S, T]
    val[s,t] = sum_pq token[s,p] W[t,p,q] h[s,q] + b_comp[t]
    act      = tanh(val)
    delta    = w_red[s] * (act[s,t] - tanh(b_comp)[t])
    out[i,t] = sum_s w_red[s]*tanh(b_comp)[t] + b_red
               + sum_{s: heads[s]==i} delta[s,t]

Sharding: data-parallel over batch B=8 -> one batch per NeuronCore; W and
the small vectors replicated. No collectives.

HW calibration (measured): PE matmul ~0.42ns per rhs column (fp16),
independent of out partitions; DVE ~0.55ns/col regardless of dtype (fp16
packing and fp8 DoubleRow give no real wins; fp8 DVE output HALVES DVE
rate). So: everything fp16, PE is the wall at ~235us, DVE ~176us under it,
Scalar nearly idle and takes all per-partition-scalar work.

Device algorithm per core (all matmuls fp16 in / f32 psum accum):
  MM1 (PE):  A[q, (i,s)] = W_ti[p,q].T @ tokT[p, s]  (t-pair i=0,1)
  TT  (DVE): Z = A * hT   (f32 PSUM in, fp16 out; the only DVE pass)
  MM2 (PE):  valT[t, s] += E_t.T @ Z_i  (staircase ones-column selector)
  ACT (Scalar): actT = tanh(valT + b_comp[t]); dT = actT - tanh(b_comp)
  dj = DMA-xbar transpose of dT (sync/gpsimd queues); dw = w_red[j] * dj
  (Scalar per-partition scale).
  MM3 (PE):  outT[t,i] += dw_j.T @ MT_j with host-baked one-hot MT.
  Chunks 0-2 of outT accumulate INLINE during the main loop (3 spare PSUM
  banks); chunk 3 runs in a short tail. outT += base (Scalar); DMA out.
Host transposes outT -> out per batch at gather time.
"""

import os
from contextlib import ExitStack

import numpy as np

import concourse.bass as bass
import concourse.tile as tile
from concourse import bacc, mybir
from concourse.bass_utils import run_bass_kernel_spmd

B, S, T = 8, 2048, 128
P = 128
N_CORES = 8
NST = S // P      # 16 s-tiles of 128
NSG = S // 512    # 4 s-groups of 512
NPAIR = T // 2    # 64 t-pairs
F32 = mybir.dt.float32
F16 = mybir.dt.float16
AF = mybir.ActivationFunctionType
ALU = mybir.AluOpType

_NC_CACHE = {}


def build_nc():
    nc = bacc.Bacc("TRN2", target_bir_lowering=False, debug=False,
                   num_devices=N_CORES)

    tokT_d = nc.dram_tensor("tokT", [P, S], F16, kind="ExternalInput").ap()
    w16_d = nc.dram_tensor("w16", [P, T * T], F16, kind="ExternalInput").ap()
    mts_d = nc.dram_tensor("mts", [S, S], F16, kind="ExternalInput").ap()
    bcompT_d = nc.dram_tensor("bcompT", [T, 1], F32, kind="ExternalInput").ap()
    nbasev_d = nc.dram_tensor("nbasev", [T, 1], F32, kind="ExternalInput").ap()
    baseT_d = nc.dram_tensor("baseT", [T, 1], F32, kind="ExternalInput").ap()
    wred_d = nc.dram_tensor("wred", [P, NST], F32, kind="ExternalInput").ap()
    outT_d = nc.dram_tensor("outT", [T, S], F32, kind="ExternalOutput").ap()

    with tile.TileContext(nc) as tc:
        _body(tc, tokT_d, w16_d, mts_d, bcompT_d, nbasev_d, baseT_d,
              wred_d, outT_d)
    nc.compile()
    return nc


def _body(tc, tokT_d, w16_d, mts_d, bcompT_d, nbasev_d, baseT_d,
          wred_d, outT_d):
    nc = tc.nc
    with ExitStack() as ctx:
        const = ctx.enter_context(tc.tile_pool(name="const", bufs=1))
        zpool = ctx.enter_context(tc.tile_pool(name="zpool", bufs=10))
        a16p = ctx.enter_context(tc.tile_pool(name="a16p", bufs=8))
        spool = ctx.enter_context(tc.tile_pool(name="spool", bufs=2))
        djp = ctx.enter_context(tc.tile_pool(name="djp", bufs=3))
        dwp = ctx.enter_context(tc.tile_pool(name="dwp", bufs=1))

        # staircase first: Q must be ready before the first MM2, and gpsimd
        # engine instructions queue behind its DMA drains otherwise.
        Q = const.tile([P, 2 * P - 1], F16)
        nc.gpsimd.memset(Q[:], 0.0)
        nc.gpsimd.memset(Q[:, P - 1: P], 1.0)

        # tokT on the (empty) sync queue so hT2+MM1 start immediately;
        # the 4MB of W alone on gpsimd, chunk 0 first.
        tokTs = []
        for g in range(NSG):
            tokT_g = const.tile([P, 512], F16, tag=f"tokT{g}", name=f"tokT{g}")
            nc.sync.dma_start(out=tokT_g[:],
                              in_=tokT_d[:, 512 * g: 512 * (g + 1)])
            tokTs.append(tokT_g)
        w_tiles = []
        for wc in range(8):
            wt = const.tile([P, 2048], F16, tag=f"w{wc}", name=f"w{wc}")
            nc.gpsimd.dma_start(out=wt[:],
                                in_=w16_d[:, 2048 * wc: 2048 * (wc + 1)])
            w_tiles.append(wt)

        # small consts on sync queue (tiny, land immediately)
        bcompT_sb = const.tile([T, 1], F32)
        nc.sync.dma_start(out=bcompT_sb[:], in_=bcompT_d[:])
        nbasev_sb = const.tile([T, 1], F32)
        nc.sync.dma_start(out=nbasev_sb[:], in_=nbasev_d[:])
        baseT_sb = const.tile([T, 1], F32)
        nc.sync.dma_start(out=baseT_sb[:], in_=baseT_d[:])
        wred_sb = const.tile([P, NST], F32)
        nc.sync.dma_start(out=wred_sb[:], in_=wred_d[:])

        # one-hot scatter matrices (host-baked): MT[j, i] = (heads[j] == i).
        # First 4 loaded up front (needed by inline MM3 after group 0);
        # the rest issue inside the loop to spread HBM traffic.
        mts = []
        for j in range(NST):
            mt_j = const.tile([P, S], F16, tag=f"mt{j}", name=f"mt{j}")
            mts.append(mt_j)
        for j in range(4):
            nc.sync.dma_start(out=mts[j][:], in_=mts_d[P * j: P * (j + 1), :])

        hT2s = []
        for g in range(NSG):
            hT2 = const.tile([P, 1024], F16, tag=f"hT2_{g}", name=f"hT2_{g}")
            nc.scalar.activation(hT2[:, 0:512], tokTs[g][:], AF.Tanh)
            nc.scalar.activation(hT2[:, 512:1024], tokTs[g][:], AF.Tanh)
            hT2s.append(hT2)

        # ---- main loop: half-pair granularity for deep pipelining ----
        # Chain per half: MM1 (216ns) -> A16 downcast (Scalar ~550) or direct
        # -> TT (DVE ~350/590) -> MM2 (216, V-accumulate serialized).
        # 6 half-A PSUM bufs (6 banks) + V x2 (2 banks) = 8 banks.
        outT_sb = const.tile([P, S], F32)
        dws = []
        with tc.tile_pool(name="psumA", bufs=6, space="PSUM") as psumA, \
             tc.tile_pool(name="psumV", bufs=2, space="PSUM") as psumV:
            for g in range(NSG):
                hT2 = hT2s[g]
                V = psumV.tile([P, 512], F32, space="PSUM", tag="V", name="V")
                for tp in range(NPAIR):
                    for i in range(2):
                        t = 2 * tp + i
                        A = psumA.tile([P, 512], F32, space="PSUM", tag="A",
                                       name="A")
                        w_t = w_tiles[t // 16][:, T * (t % 16): T * (t % 16 + 1)]
                        nc.tensor.matmul(A[:], lhsT=w_t, rhs=tokTs[g][:],
                                         start=True, stop=True)
                        Z = zpool.tile([P, 512], F16, tag="Z", name="Z")
                        if tp % 4 != 3:
                            # Scalar downcast to fp16 SBUF: DVE reads PSUM
                            # f32 at half rate, so 3/4 of halves go through
                            # the otherwise-idle Scalar engine.
                            A16 = a16p.tile([P, 512], F16, tag="A16",
                                            name="A16")
                            nc.scalar.activation(A16[:], A[:], AF.Copy)
                            nc.vector.tensor_tensor(out=Z[:], in0=A16[:],
                                                    in1=hT2[:, 0:512],
                                                    op=ALU.mult)
                        else:
                            nc.vector.tensor_tensor(out=Z[:], in0=A[:],
                                                    in1=hT2[:, 0:512],
                                                    op=ALU.mult)
                        nc.tensor.matmul(V[:],
                                         lhsT=Q[:, P - 1 - t: 2 * P - 1 - t],
                                         rhs=Z[:], start=(tp == 0 and i == 0),
                                         stop=(tp == NPAIR - 1 and i == 1))
                actT = spool.tile([P, 512], F16, tag="actT", name="actT")
                nc.scalar.activation(actT[:], V[:], AF.Tanh, bias=bcompT_sb[:])
                dT = spool.tile([P, 512], F16, tag="dT", name="dT")
                nc.vector.tensor_scalar_add(dT[:], actT[:], nbasev_sb[:])
                for k in range(4):
                    j = 4 * g + k
                    dj = djp.tile([P, P], F16, tag="dj", name="dj")
                    nc.sync.dma_start_transpose(out=dj[:],
                                                in_=dT[:, P * k: P * (k + 1)])
                    dw_j = dwp.tile([P, P], F16, tag=f"dw{j}", name=f"dw{j}")
                    nc.vector.tensor_scalar_mul(dw_j[:], dj[:],
                                                wred_sb[:, j: j + 1])
                    dws.append(dw_j)
                    # spread the remaining one-hot loads through the loop
                    if g < 3:
                        jn = 4 * (g + 1) + k
                        nc.sync.dma_start(out=mts[jn][:],
                                          in_=mts_d[P * jn: P * (jn + 1), :])

        # ---- scatter tail: outT[t, i] = sum_j dw[j, t] * MT[j, i] + base ----
        with tc.tile_pool(name="psumO", bufs=1, space="PSUM") as psumO:
            for c in range(4):
                OT = psumO.tile([P, 512], F32, space="PSUM", tag=f"OT{c}",
                                name=f"OT{c}")
                for j in range(NST):
                    nc.tensor.matmul(OT[:], lhsT=dws[j][:],
                                     rhs=mts[j][:, 512 * c: 512 * (c + 1)],
                                     start=(j == 0), stop=(j == NST - 1))
                cs = slice(512 * c, 512 * (c + 1))
                nc.vector.tensor_scalar_add(outT_sb[:, cs], OT[:],
                                            baseT_sb[:])
                nc.sync.dma_start(out=outT_d[:, cs], in_=outT_sb[:, cs])


def _prep_inputs(token_embeddings, dep_heads, W_comp, b_comp, w_red, b_red):
    """Host-side sharding + layout prep. One in_map per core (= per batch)."""
    token = np.asarray(token_embeddings, np.float32)
    heads = np.asarray(dep_heads, np.int32)
    W = np.asarray(W_comp, np.float32)
    w16 = np.ascontiguousarray(
        W.transpose(1, 0, 2).reshape(P, T * T).astype(np.float16))
    bc = np.asarray(b_comp, np.float32)
    bcompT = np.ascontiguousarray(bc.reshape(T, 1))
    basev = np.tanh(bc)
    nbasev = np.ascontiguousarray(-basev.reshape(T, 1)).astype(np.float32)
    w = np.asarray(w_red, np.float32)[0]
    baseT = (w.sum() * basev + np.asarray(b_red, np.float32)[0])
    baseT = np.ascontiguousarray(baseT.reshape(T, 1)).astype(np.float32)
    wred = np.ascontiguousarray(w.reshape(NST, P).T)

    in_maps = []
    for b in range(B):
        mts = np.zeros((S, S), np.float16)
        mts[np.arange(S), heads[b]] = 1.0
        in_maps.append({
            "tokT": np.ascontiguousarray(token[b].T.astype(np.float16)),
            "w16": w16,
            "mts": mts,
            "bcompT": bcompT,
            "nbasev": nbasev,
            "baseT": baseT,
            "wred": wred,
        })
    return in_maps


def kernel(**inputs):
    if "nc" not in _NC_CACHE:
        _NC_CACHE["nc"] = build_nc()
    nc = _NC_CACHE["nc"]
    in_maps = _prep_inputs(
        inputs["token_embeddings"], inputs["dep_heads"], inputs["W_comp"],
        inputs["b_comp"], inputs["w_red"], inputs["b_red"])
    res = run_bass_kernel_spmd(nc, in_maps, core_ids=list(range(N_CORES)))
    out = np.empty((B, S, T), np.float32)
    for b in range(B):
        out[b] = res.results[b]["outT"].T
    return out
